# revision 46
# baseline (speedup 1.0000x reference)
"""Multi-head dot-product attention with prefix KV, on 8 trn2 NeuronCores.

Sharding: batch (2) x head-groups (4 groups of 4 heads) = 8 cores.
Each core computes q/k/v projections for its 4 heads, flash-style
attention (scores kept transposed: [kv, L] so no on-device transposes
are needed), and a partial out-projection [E, L]; the host sums the 4
head-group partials per batch and transposes back.

Layout/schedule notes (vs 304us fp32 / 195us first bf16 version):
  - all matmul operands are bf16 (PSUM accumulation stays fp32): halves
    LDWEIGHTS time and input DMA, and enables the fast DVE modes.
  - all DRAM inputs are pre-tiled on the host so every DMA source is
    contiguous per partition (fat descriptors).  Descriptor GENERATION on
    the Sync engine (~5ns each) gates the startup, not DMA bandwidth:
    thin 1KB-row layouts cost ~5us per big tensor before data even moves.
    First projection matmuls additionally gate on 4-ec half-loads.
  - ~72 tiny warmup matmuls run during the ~8us DMA-boot window so the
    PE p-state ramp (0.65 -> 2.4GHz needs ~3us of continuous busy) is
    done before real work lands.
  - kv axis padded to 2176 = 17*128: chunk 0 = [prefix(64) | dead(64)],
    chunks 1..16 = kv positions.  Dead columns are killed with a
    per-partition -1e10 bias on the chunk-0 exp.
  - causal q-trim per (group, chunk) with one exception: the second
    chunk of a score pair-batch starts at its 512 boundary so the single
    per-batch exp never reads unwritten PSUM (the extra columns are
    harmless garbage the ctx matmul never consumes).
  - attention is software-pipelined one batch ahead: batch b+1's score
    matmuls sit between exp(b) and ctx(b) in the PE stream, so the
    in-order PE queue always holds real work covering the ACT latency.
  - softmax runs without max subtraction (scores are O(1)); denominator
    comes free as a ones-column in the v weights (M=65 ctx matmul).
    Groups 0..2 gather denom rows to partitions {0,32,64,96}, run one
    fast-approx DVE reciprocal, and broadcast with K=1 bf16 matmuls —
    the finish is split into parts fed ONE PER BATCH of the next group,
    so the PE reaches each broadcast a full batch after its DVE producer
    and never stalls.  Group 3 finishes per head, deferred one head
    ([1,3,2,0] order; par==1 heads and their SBUF DMA hop first, the
    tail head emits directly; the raw denom row is cast to bf16 so the
    broadcast matmul runs at 1 cycle/row instead of fp32's 4).
  - q/k projections use N=512 matmuls; projection units (atomic 8-matmul
    accumulation groups) and out-projection units interleave between the
    score and ctx matmuls as PE filler.  outproj(g) is emitted inside
    group g+1 right after finish(g); outproj(2) is rationed across group
    3's heads with 4 units held back so the PE still has queued work
    while the tail denb chain runs; outproj(3) is the tail (6-deep
    staging ring).
"""

import numpy as np
import ml_dtypes

BF16 = ml_dtypes.bfloat16

B, LQ, LKV, E, H, D, P = 2, 2048, 2048, 1024, 16, 64, 64
NCORES = 8
HGROUPS = 4          # head groups (cores per batch)
HPC = H // HGROUPS   # heads per core = 4
KVPAD = 128 + LKV    # 2176
NCH = KVPAD // 128   # 17 chunks
NG = LQ // 512       # 4 L-groups of 512
NEG = -1.0e10

_CACHE = {}


def _build_module(plan, debug_taps=False):
    """Build the single-core Bass module (same program for all 8 cores)."""
    import concourse.bass as bass
    import concourse.tile as tile
    import concourse.mybir as mybir
    from concourse import bacc
    from contextlib import ExitStack

    f32 = mybir.dt.float32
    bf16 = mybir.dt.bfloat16
    Exp = mybir.ActivationFunctionType.Exp

    chunks = plan["chunks"]        # g -> [c...] ascending, 0 first
    qlo = plan["qlo"]              # (g,c) -> valid-q start col (0..511)
    win = plan["win"]              # (g,c) -> (mlo, mhi, tile_idx) or absent
    ntiles = plan["ntiles"]

    nc = bacc.Bacc("TRN2", target_bir_lowering=False, debug=False,
                   enable_asserts=False, num_devices=NCORES)

    # inputs are pre-tiled on the host so every DMA source is contiguous
    # per partition (fat descriptors: descriptor GENERATION on the Sync
    # engine, ~5ns each, is what gates the startup — not DMA bandwidth)
    xqT_d = nc.dram_tensor("xqT", [128, NG, 8, 512], bf16,
                           kind="ExternalInput").ap()
    xkvT_d = nc.dram_tensor("xkvT", [128, NG, 8, 512], bf16,
                            kind="ExternalInput").ap()
    wq_d = nc.dram_tensor("wq", [128, 8, 256], bf16, kind="ExternalInput").ap()
    wk_d = nc.dram_tensor("wk", [128, 8, 256], bf16, kind="ExternalInput").ap()
    wv_d = nc.dram_tensor("wv", [128, 8, 256], bf16, kind="ExternalInput").ap()
    wo_d = nc.dram_tensor("wo", [128, 2, 1024], bf16, kind="ExternalInput").ap()
    kprefT_d = nc.dram_tensor("kprefT", [128, 2, 128], bf16,
                              kind="ExternalInput").ap()
    vpref_d = nc.dram_tensor("vpref", [128, HPC, D], bf16, kind="ExternalInput").ap()
    if ntiles:
        maskblk_d = nc.dram_tensor("maskblk", [128, ntiles, 128], bf16,
                                   kind="ExternalInput").ap()
    outT_d = nc.dram_tensor("outT", [E, LQ], bf16, kind="ExternalOutput").ap()

    with tile.TileContext(nc) as tc, ExitStack() as stk:
        pers = stk.enter_context(tc.tile_pool(name="pers", bufs=1))

        def ptile(shape, name, dt=bf16):
            return pers.tile(shape, dt, tag=name, name=name)

        wq_sb = ptile([128, 8, 256], "wq_sb")
        wk_sb = ptile([128, 8, 256], "wk_sb")
        wv_sb = ptile([128, 8, 256], "wv_sb")
        wo_sb = ptile([128, 2, 1024], "wo_sb")
        # per-L-group tensors (512 wide); K prefix is its own [128,128] tile
        QTS = [[ptile([128, 512], f"QT{i}g{g}") for g in range(NG)] for i in range(2)]
        KTS = [[ptile([128, 512], f"KT{i}g{g}") for g in range(NG)] for i in range(2)]
        KPR = [ptile([128, 128], f"KP{i}") for i in range(2)]
        VTS = [ptile([128, HPC, 65], f"VT{c}") for c in range(NCH)]
        CTXT = [[ptile([128, 512], f"CTXT{i}g{g}") for g in range(NG)]
                for i in range(2)]
        cb0 = ptile([128, 1], "cb0", f32)
        ones_col = ptile([128, 64], "ones_col")
        mtall = ptile([128, max(ntiles, 1), 128], "mtall")

        def kslice(hc, c):
            if c == 0:
                return KPR[hc]
            g, off = (c - 1) // 4, 128 * ((c - 1) % 4)
            return KTS[hc][g][:, off:off + 128]

        xio = stk.enter_context(tc.tile_pool(name="xio", bufs=2))

        def proj_load(g, xq_t=None, xkv_t=None, halves=1):
            """x loads; `halves=2` splits per 4-ec half so the first
            projection matmuls gate on 0.5MB instead of 2MB (startup)."""
            if xq_t is None:
                xq_t = xio.tile([128, 8, 512], bf16, tag="xq", bufs=2, name="xq_t")
                xkv_t = xio.tile([128, 8, 512], bf16, tag="xkv", bufs=2,
                                 name="xkv_t")
            step = 8 // halves
            for t, t_d in ((xq_t, xqT_d), (xkv_t, xkvT_d)):
                for e0 in range(0, 8, step):
                    nc.sync.dma_start(out=t[:, e0:e0 + step, :],
                                      in_=t_d[:, g, e0:e0 + step, :])
            return xq_t, xkv_t

        # startup DMAs ordered by first consumption (q-t0 wq+xq, k-t0
        # wk+xkv, v xkv+wv); every source is per-partition contiguous so
        # descgen is ~128 descriptors per load
        xq0 = xio.tile([128, 8, 512], bf16, tag="xq", bufs=2, name="xq_t")
        xkv0 = xio.tile([128, 8, 512], bf16, tag="xkv", bufs=2, name="xkv_t")
        for e0 in (0, 4):
            nc.sync.dma_start(out=wq_sb[:, e0:e0 + 4, :], in_=wq_d[:, e0:e0 + 4, :])
            nc.sync.dma_start(out=xq0[:, e0:e0 + 4, :], in_=xqT_d[:, 0, e0:e0 + 4, :])
        nc.sync.dma_start(out=wk_sb, in_=wk_d)
        for e0 in (0, 4):
            nc.sync.dma_start(out=xkv0[:, e0:e0 + 4, :],
                              in_=xkvT_d[:, 0, e0:e0 + 4, :])
        nc.sync.dma_start(out=wv_sb, in_=wv_d)
        ld0 = (xq0, xkv0)

        nc.vector.memset(ones_col, 1.0)
        nc.vector.memset(cb0[0:64, :], 0.0)
        nc.vector.memset(cb0[64:128, :], NEG)
        for c in range(NCH):
            nc.vector.memset(VTS[c][:, :, 64:65], 1.0)
        for hc in range(2):
            nc.sync.dma_start(out=KPR[hc], in_=kprefT_d[:, hc, :])
        nc.sync.dma_start(out=VTS[0][:, :, 0:D], in_=vpref_d)
        if ntiles:
            nc.sync.dma_start(out=mtall, in_=maskblk_d)
        nc.sync.dma_start(out=wo_sb, in_=wo_d)

        attps = stk.enter_context(tc.tile_pool(name="att_ps", bufs=1, space="PSUM"))
        attsb = stk.enter_context(tc.tile_pool(name="att_sb", bufs=1))
        pjps = stk.enter_context(tc.tile_pool(name="pj_ps", bufs=1, space="PSUM"))

        # PE p-state warmup: the clock ramps only while the PE is busy, so
        # burn the DMA-wait window (engines boot ~7us, first real matmul's
        # data lands ~9us) on dummy matmuls; real work then starts warm.
        wps = pjps.tile([128, 512], f32, tag="pj", bufs=2, name="warm")
        for _ in range(72):
            nc.tensor.matmul(wps[0:64, 0:64], lhsT=ones_col[:, 0:64],
                             rhs=ones_col[:, 0:64], start=True, stop=True)

        def proj_units(g, loaded):
            """q/k/v projection for L-group g as a sequence of 4-matmul
            quanta (yield between quanta so attention can interleave)."""
            xq_t, xkv_t = loaded
            # each unit is atomic: its 8-matmul PSUM accumulation group must
            # close before anything else can allocate from the pj ring
            # q units first (both need only wq+xq, which arrive first at
            # startup); k units follow while wk/xkv are still streaming in
            for w_sb, x_t, dst in ((wq_sb, xq_t, QTS), (wk_sb, xkv_t, KTS)):
                for t in range(2):
                    ps = pjps.tile([128, 512], f32, tag="pj", bufs=2, name="ps_p")
                    for ec in range(8):
                        nc.tensor.matmul(
                            ps, lhsT=w_sb[:, ec, 128 * t:128 * t + 128],
                            rhs=x_t[:, ec, :], start=(ec == 0), stop=(ec == 7))
                    nc.vector.tensor_copy(out=dst[t][g], in_=ps)
                    yield
            for sub in range(4):
                ps = pjps.tile([128, 512], f32, tag="pj", bufs=2, name="ps_v")
                for ec in range(8):
                    nc.tensor.matmul(
                        ps[:, 0:256], lhsT=xkv_t[:, ec, 128 * sub:128 * sub + 128],
                        rhs=wv_sb[:, ec, :], start=(ec == 0), stop=(ec == 7))
                nc.vector.tensor_copy(
                    out=VTS[4 * g + sub + 1][:, :, 0:D],
                    in_=ps[:, 0:256].rearrange("p (h d) -> p h d", h=HPC))
                yield

        op_n = [0]

        def outproj_units(g):
            """out-projection for L-group g: 8 units of (2 matmuls + copy)."""
            gl = 512 * g
            for et in range(8):
                ops = pjps.tile([128, 512], f32, tag="pj", bufs=2, name="ops")
                for hc in range(2):
                    nc.tensor.matmul(
                        ops, lhsT=wo_sb[:, hc, 128 * et:128 * et + 128],
                        rhs=CTXT[hc][g], start=(hc == 0), stop=(hc == 1))
                ot = attsb.tile([128, 512], bf16, tag="ostage", bufs=6, name="ot")
                # PSUM->SBUF copy on DVE (ACT is saturated by the softmax
                # exps mid-run); the last group's units alternate onto ACT,
                # which is idle during the tail
                if g == NG - 1 and op_n[0] % 2 == 0:
                    nc.scalar.copy(ot, ops)
                else:
                    nc.vector.tensor_copy(out=ot, in_=ops)
                op_n[0] += 1
                nc.sync.dma_start(
                    out=outT_d[128 * et:128 * et + 128, gl:gl + 512], in_=ot)
                yield

        class FillerQ:
            """Two queues of generators (proj must flush at group
            boundaries, outproj carries over); step() advances one quantum,
            preferring proj."""
            def __init__(self):
                self.proj = []
                self.op = []

            @staticmethod
            def _step_one(q):
                while q:
                    try:
                        next(q[0])
                        return True
                    except StopIteration:
                        q.pop(0)
                return False

            def step(self, n=1):
                for _ in range(n):
                    if not self._step_one(self.proj):
                        self._step_one(self.op)

            def flush_proj(self):
                while self._step_one(self.proj):
                    pass

            def drain_all(self):
                self.flush_proj()
                while self._step_one(self.op):
                    pass

        fill = FillerQ()

        def denb_head(g, h, cx):
            """Per-head denominator finish (last group): partition-broadcast
            the raw denom row via a K=1 matmul, fast reciprocal (at
            partition offset 0 — the custom DVE op needs that), CTXT
            scaling on DVE."""
            hc, par = h // 2, h % 2
            # bf16 the raw denom row first: the K=1 broadcast matmul then
            # runs at 1 cycle/row instead of fp32's 4
            db = attsb.tile([1, 512], bf16, tag="db", bufs=2, name="db")
            nc.vector.tensor_copy(out=db, in_=cx[64:65, :])
            bc_ps = pjps.tile([128, 512], f32, tag="pj", bufs=2, name="bc_ps")
            nc.tensor.matmul(bc_ps[0:64, :], lhsT=ones_col[0:1, :],
                             rhs=db, start=True, stop=True)
            rcs = attsb.tile([64, 512], f32, tag="rcs", bufs=2, name="rcs")
            nc.vector.reciprocal_approx_fast(out=rcs, in_=bc_ps[0:64, :])
            if par == 0:
                nc.vector.tensor_mul(CTXT[hc][g][0:64, :], cx[0:64, :], rcs)
            else:
                st = attsb.tile([64, 512], bf16, tag="stage", bufs=2, name="st")
                nc.vector.tensor_mul(st, cx[0:64, :], rcs)
                nc.sync.dma_start(out=CTXT[hc][g][64:128, :], in_=st)

        def emit_scores(g, h, batch):
            """Score matmuls + exp for one chunk batch; returns the pr tile."""
            hc, prow = h // 2, 64 * (h % 2)
            sc = attps.tile([128, 1024], f32, tag="sc", bufs=2, name=f"sc{h}")
            for j, c in enumerate(batch):
                q0 = qlo[(g, c)]
                # second chunk of a pair starts at its 512 boundary (q0=0)
                # even when causally trimmable: the few extra columns are
                # harmless garbage scores and keep the whole exp range of
                # the single per-batch activation initialized
                m0 = q0 if j == 0 else 0
                nc.tensor.matmul(
                    sc[:, 512 * j + m0:512 * j + 512],
                    lhsT=kslice(hc, c)[prow:prow + 64, :],
                    rhs=QTS[hc][g][prow:prow + 64, m0:512],
                    start=True, stop=True)
            pr = attsb.tile([128, 1024], bf16, tag="pr", bufs=4, name=f"pr{h}")
            e0 = qlo[(g, batch[0])]
            we = 512 * len(batch)
            if batch[0] == 0:
                nc.scalar.activation(pr[:, e0:we], sc[:, e0:we], Exp,
                                     bias=cb0[:, 0:1])
            else:
                nc.scalar.activation(pr[:, e0:we], sc[:, e0:we], Exp)
            return pr

        def attn_group(g, head_hook=None, pending_cb=None):
            """Score/softmax/ctx for group g, software-pipelined one batch
            ahead: batch b+1's score matmuls sit between exp(b) and ctx(b)
            in the PE stream, so the in-order PE queue always has real work
            covering the softmax (ACT) latency.  The denominator finish is
            entirely off-PE (GpSimd broadcast + DVE reciprocal/scale), so
            group boundaries are seamless for the PE."""
            cs = chunks[g]
            batches = [[cs[0]]] + [cs[1 + i:3 + i] for i in range(0, len(cs) - 1, 2)]
            nbat = len(batches)
            last = (g == NG - 1)
            ctxs = {}
            if not last:
                denoms4 = attsb.tile([97, 512], f32, tag="den4", bufs=2,
                                     name="denoms4")
                nc.vector.memset(denoms4, 1.0)
            # last group runs par==1 heads (whose CTXT write needs an extra
            # SBUF->SBUF DMA hop) first, so the tail head finishes directly
            pend_denb = None
            for hi, h in enumerate([1, 3, 2, 0] if last else range(HPC)):
                if head_hook is not None:
                    head_hook()
                ctx_ps = attps.tile([65, 512], f32, tag="ctx", bufs=2,
                                    name=f"ctx{h}")
                prs = {0: emit_scores(g, h, batches[0])}
                if pend_denb is not None:
                    # previous head's denb, emitted one head late so its
                    # broadcast matmul never stalls the PE queue head
                    pend_denb()
                    pend_denb = None
                for bi, batch in enumerate(batches):
                    if bi + 1 < nbat:
                        prs[bi + 1] = emit_scores(g, h, batches[bi + 1])
                    fill.step(1)
                    if pending_cb and not (hi == 0 and bi == 0):
                        # one previous-group finish part per batch: the PE
                        # reaches each broadcast a full batch after its DVE
                        # producer was emitted, so it never stalls
                        pending_cb.pop(0)()
                    pr = prs.pop(bi)
                    for j, c in enumerate(batch):
                        for mlo, mhi, ti in win.get((g, c), ()):
                            nc.vector.tensor_mul(
                                pr[:, 512 * j + mlo:512 * j + mhi],
                                pr[:, 512 * j + mlo:512 * j + mhi],
                                mtall[:, ti, 0:mhi - mlo])
                    for j, c in enumerate(batch):
                        q0 = qlo[(g, c)]
                        nc.tensor.matmul(
                            ctx_ps[:, q0:512],
                            lhsT=VTS[c][:, h, :],
                            rhs=pr[:, 512 * j + q0:512 * j + 512],
                            start=(bi == 0 and j == 0),
                            stop=(bi == nbat - 1 and j == len(batch) - 1))
                    fill.step(1)
                # copy ctx+denom to SBUF to release the PSUM bank
                ctxs[h] = attsb.tile([65, 512], f32, tag="ctxs", bufs=8,
                                     name=f"ctxs{h}")
                nc.vector.tensor_copy(out=ctxs[h], in_=ctx_ps)
                if last:
                    # last group: per-head denominator finish (deferred one
                    # head; the tail head emits immediately)
                    if h == 0:
                        denb_head(g, h, ctxs[h])
                    else:
                        pend_denb = (lambda hh=h: denb_head(g, hh, ctxs[hh]))
                else:
                    # gather the denom row into denoms4 at partition 32h
                    # for the batched per-group reciprocal
                    nc.sync.dma_start(out=denoms4[32 * h:32 * h + 1, :],
                                      in_=ctxs[h][64:65, :])
            while pending_cb:
                pending_cb.pop(0)()
            if last:
                return None

            # batched denominator finish, returned as SPREAD parts: one
            # reciprocal over the four gathered denom rows, then per head a
            # K=1 broadcast matmul + DVE scale.  The caller feeds one part
            # per batch of the next group so the in-order PE queue reaches
            # each broadcast well after its DVE producer.
            state = {}

            def recip_part():
                rc4 = attsb.tile([97, 512], f32, tag="rc4", bufs=2, name="rc4")
                nc.vector.reciprocal_approx_fast(out=rc4, in_=denoms4)
                rc4b = attsb.tile([97, 512], bf16, tag="rc4b", bufs=2,
                                  name="rc4b")
                nc.vector.tensor_copy(out=rc4b, in_=rc4)
                state["rc4b"] = rc4b

            def bc_part(h):
                hc, par = h // 2, h % 2
                rc4b = state["rc4b"]
                bc_ps = pjps.tile([128, 512], f32, tag="pj", bufs=2,
                                  name="bc_ps")
                nc.tensor.matmul(bc_ps[0:64, :],
                                 lhsT=ones_col[32 * h:32 * h + 1, :],
                                 rhs=rc4b[32 * h:32 * h + 1, :],
                                 start=True, stop=True,
                                 tile_position=(32 * h, 0))
                if par == 0:
                    nc.vector.tensor_mul(CTXT[hc][g][0:64, :],
                                         ctxs[h][0:64, :], bc_ps[0:64, :])
                else:
                    st = attsb.tile([64, 512], bf16, tag="stage", bufs=2,
                                    name="st")
                    nc.vector.tensor_mul(st, ctxs[h][0:64, :],
                                         bc_ps[0:64, :])
                    nc.sync.dma_start(out=CTXT[hc][g][64:128, :], in_=st)
            return [recip_part] + [lambda h=h: bc_part(h) for h in range(HPC)]

        # schedule: proj(0) upfront; during attn(g) the filler queue holds
        # proj(g+1) plus outproj(g-1) (whose CTXT finished, off-PE, at the
        # end of attn(g-1)); proj leftovers flush at the group boundary
        # (they gate attn(g+1)); op leftovers carry over and also cover the
        # final denb chain before the outproj(3) tail.
        def take(gen, n):
            def taken():
                for _ in range(n):
                    try:
                        next(gen)
                    except StopIteration:
                        return
                    yield
            return taken()

        for _ in proj_units(0, ld0):
            pass
        pending = []
        for g in range(NG):
            if g + 1 < NG:
                ld = proj_load(g + 1)
                fill.proj.append(proj_units(g + 1, ld))
            if g == NG - 1:
                # ration outproj(2): 2 units at the 2nd/3rd head starts
                # (after the finish(2) parts drained), 4 held back so the
                # PE still has work queued behind the final ctx matmuls
                # while the tail denb chain runs
                op2 = outproj_units(g - 1)
                feed = iter([0, 2, 2, 0])

                def hook():
                    n = next(feed)
                    if n:
                        fill.op.append(take(op2, n))
                attn_group(g, head_hook=hook, pending_cb=pending)
                fill.op.append(op2)
            else:
                parts = attn_group(g, pending_cb=pending)
                fill.flush_proj()
                pending = parts
                if g + 1 < NG - 1:
                    # after the last finish part: outproj(g) becomes
                    # available (emission order guarantees op reads CTXT
                    # after the finish writes it)
                    pending = parts + [
                        lambda g=g: fill.op.append(outproj_units(g))]
        fill.drain_all()
        for _ in outproj_units(NG - 1):
            pass

    nc.compile()
    return nc


def _make_plan(mask):
    """Block plan from the actual mask (union over batches -> one SPMD plan).

    For each (q-group g, kv-chunk c) computes:
      - inclusion (any valid element),
      - qlo: first q column (within the group's 512) with any valid kv,
      - the mixed window [mlo, mhi) of q columns that need an elementwise
        mask multiply, with deduped [128, mhi-mlo<=128...] tiles.
    """
    m = np.asarray(mask[:, 0])                       # [B, LQ, LKV] bool
    chunks, qlo, wins = [], {}, {}
    uniq, order = {}, []                             # content-hash -> idx
    for g in range(NG):
        cl = [0]
        qlo[(g, 0)] = 0
        for c in range(1, NCH):
            blk = m[:, 512 * g:512 * g + 512, 128 * (c - 1):128 * c]  # [B,512,128]
            anyk = blk.any(axis=2)                   # [B, 512]
            if not anyk.any():
                continue
            cl.append(c)
            valid_cols = anyk.any(axis=0)            # union over batches
            q0 = int(np.argmax(valid_cols))
            qlo[(g, c)] = q0
            allk = blk.all(axis=2).all(axis=0)       # [512] all-valid cols
            mixed = valid_cols & ~allk
            if mixed.any():
                lo = int(np.argmax(mixed))
                hi = 512 - int(np.argmax(mixed[::-1]))
                wl = []
                # split into <=128-wide windows (mask tiles are [128,128])
                for mlo in range(lo, hi, 128):
                    mhi = min(mlo + 128, hi)
                    # per-batch tile content; dedup on the union key so all
                    # cores run the same program with per-core data
                    key = (mhi - mlo, blk[:, mlo:mhi, :].tobytes())
                    if key not in uniq:
                        uniq[key] = len(order)
                        order.append((g, c, mlo, mhi))
                    wl.append((mlo, mhi, uniq[key]))
                wins[(g, c)] = wl
        chunks.append(cl)
    return {"chunks": chunks, "qlo": qlo, "win": wins, "ntiles": len(order),
            "order": order}


def _prep_core_inputs(inputs, plan):
    """Per-core input dicts (8 cores: batch-major, then head-group)."""
    inputs_q = np.asarray(inputs["inputs_q"], dtype=np.float32)
    inputs_kv = np.asarray(inputs["inputs_kv"], dtype=np.float32)
    key_prefix = np.asarray(inputs["key_prefix"], dtype=np.float32)
    value_prefix = np.asarray(inputs["value_prefix"], dtype=np.float32)
    mask = np.asarray(inputs["mask"])
    Wq = np.asarray(inputs["Wq"], dtype=np.float32)
    Wk = np.asarray(inputs["Wk"], dtype=np.float32)
    Wv = np.asarray(inputs["Wv"], dtype=np.float32)
    Wo = np.asarray(inputs["Wo"], dtype=np.float32)

    def tile_x(x):
        # [L, E] -> [E, L] -> [128p, NG, 8ec, 512]: per-partition contiguous
        xT = x.T.reshape(8, 128, NG, 512).transpose(1, 2, 0, 3)
        return np.ascontiguousarray(xT.astype(BF16))

    def tile_w(w):
        # [E, 256] -> [128p, 8ec, 256]
        return np.ascontiguousarray(
            w.reshape(8, 128, HPC * D).transpose(1, 0, 2).astype(BF16))

    xT = [tile_x(inputs_q[b]) for b in range(B)]
    xkT = [tile_x(inputs_kv[b]) for b in range(B)]

    maskblks = []
    ntiles = plan["ntiles"]
    for b in range(B):
        mb = np.zeros((max(ntiles, 1), 128, 128), np.float32)
        for i, (g, c, mlo, mhi) in enumerate(plan["order"]):
            mb[i, :, 0:mhi - mlo] = mask[
                b, 0, 512 * g + mlo:512 * g + mhi,
                128 * (c - 1):128 * c].T.astype(np.float32)
        # -> [128p, ntiles, 128]: per-partition contiguous
        maskblks.append(np.ascontiguousarray(
            mb.transpose(1, 0, 2).astype(BF16)))

    in_maps = []
    for core in range(NCORES):
        b, hg = core // HGROUPS, core % HGROUPS
        hs = slice(HPC * hg, HPC * (hg + 1))
        kpT = key_prefix[b, :, hs, :]                 # [P, 4, D]
        kpT = kpT.transpose(1, 2, 0).reshape(2, 128, P)  # [hc, (2 heads x D), P]
        kpT = np.concatenate(
            [kpT, np.zeros((2, 128, 128 - P), np.float32)], axis=2)
        im = {
            "xqT": xT[b],
            "xkvT": xkT[b],
            "wq": tile_w((Wq[:, hs, :] / np.sqrt(D)).reshape(E, HPC * D)),
            "wk": tile_w(Wk[:, hs, :].reshape(E, HPC * D)),
            "wv": tile_w(Wv[:, hs, :].reshape(E, HPC * D)),
            "wo": np.ascontiguousarray(
                Wo[hs].reshape(2, 128, E).transpose(1, 0, 2).astype(BF16)),
            "kprefT": np.ascontiguousarray(
                kpT.transpose(1, 0, 2).astype(BF16)),
            "vpref": np.ascontiguousarray(np.concatenate(
                [value_prefix[b, :, hs, :],
                 np.zeros((128 - P, HPC, D), np.float32)], axis=0).astype(BF16)),
        }
        if ntiles:
            im["maskblk"] = maskblks[b]
        in_maps.append(im)
    return in_maps


def kernel(**inputs) -> np.ndarray:
    from concourse import bass_utils

    plan = _make_plan(inputs["mask"])
    key = (tuple(tuple(c) for c in plan["chunks"]),
           tuple(sorted(plan["qlo"].items())),
           tuple(sorted((k, tuple(v)) for k, v in plan["win"].items())),
           plan["ntiles"])
    if key not in _CACHE:
        _CACHE[key] = _build_module(plan)
    nc = _CACHE[key]

    in_maps = _prep_core_inputs(inputs, plan)
    res = bass_utils.run_bass_kernel_spmd(nc, in_maps, core_ids=list(range(NCORES)))

    out = np.zeros((B, LQ, E), np.float32)
    for core in range(NCORES):
        b = core // HGROUPS
        out[b] += res.results[core]["outT"].T.astype(np.float32)
    return out



# revision 54
# speedup vs baseline: 1.2001x; 1.2001x over previous
"""Multi-head dot-product attention with prefix KV, on 8 trn2 NeuronCores.

Sharding: batch (2) x head-groups (4 groups of 4 heads) = 8 cores.
Each core computes q/k/v projections for its 4 heads, flash-style
attention (scores kept transposed: [kv, L] so no on-device transposes
are needed), and a partial out-projection [E, L]; the host sums the 4
head-group partials per batch and transposes back.

Layout/schedule notes (vs 304us fp32 / 195us first bf16 version):
  - all matmul operands are bf16 (PSUM accumulation stays fp32): halves
    LDWEIGHTS time and input DMA, and enables the fast DVE modes.
  - all DRAM inputs are pre-tiled on the host so every DMA source is
    contiguous per partition (fat descriptors).  Descriptor GENERATION on
    the Sync engine (~5ns each) gates the startup, not DMA bandwidth:
    thin 1KB-row layouts cost ~5us per big tensor before data even moves.
    First projection matmuls additionally gate on 4-ec half-loads.
  - ~72 tiny warmup matmuls run during the ~8us DMA-boot window so the
    PE p-state ramp (0.65 -> 2.4GHz needs ~3us of continuous busy) is
    done before real work lands.
  - kv axis padded to 2176 = 17*128: chunk 0 = [prefix(64) | dead(64)],
    chunks 1..16 = kv positions.  Dead columns are killed with a
    per-partition -1e10 bias on the chunk-0 exp.
  - causal q-trim per (group, chunk) with one exception: the second
    chunk of a score pair-batch starts at its 512 boundary so the single
    per-batch exp never reads unwritten PSUM (the extra columns are
    harmless garbage the ctx matmul never consumes).
  - attention is software-pipelined one batch ahead: batch b+1's score
    matmuls sit between exp(b) and ctx(b) in the PE stream, so the
    in-order PE queue always holds real work covering the ACT latency.
  - softmax runs without max subtraction (scores are O(1)); denominator
    comes free as a ones-column in the v weights (M=65 ctx matmul).
    Groups 0..2 gather denom rows to partitions {0,32,64,96}, run one
    fast-approx DVE reciprocal, and broadcast with K=1 bf16 matmuls —
    the finish is split into parts fed ONE PER BATCH of the next group,
    so the PE reaches each broadcast a full batch after its DVE producer
    and never stalls.  Group 3 finishes per head, deferred one head
    ([1,3,2,0] order; par==1 heads and their SBUF DMA hop first, the
    tail head emits directly; the raw denom row is cast to bf16 so the
    broadcast matmul runs at 1 cycle/row instead of fp32's 4).
  - q/k projections use N=512 matmuls; projection units (atomic 8-matmul
    accumulation groups) and out-projection units interleave between the
    score and ctx matmuls as PE filler.  outproj(g) is emitted inside
    group g+1 right after finish(g); outproj(2) is rationed across group
    3's heads with 4 units held back so the PE still has queued work
    while the tail denb chain runs; outproj(3) is the tail (6-deep
    staging ring).
"""

import numpy as np
import ml_dtypes

BF16 = ml_dtypes.bfloat16

B, LQ, LKV, E, H, D, P = 2, 2048, 2048, 1024, 16, 64, 64
NCORES = 8
HGROUPS = 4          # head groups (cores per batch)
HPC = H // HGROUPS   # heads per core = 4
KVPAD = 128 + LKV    # 2176
NCH = KVPAD // 128   # 17 chunks
NG = LQ // 512       # 4 L-groups of 512
NEG = -1.0e10

_CACHE = {}


def _build_module(plan, debug_taps=False):
    """Build the single-core Bass module (same program for all 8 cores)."""
    import concourse.bass as bass
    import concourse.tile as tile
    import concourse.mybir as mybir
    from concourse import bacc
    from contextlib import ExitStack

    f32 = mybir.dt.float32
    bf16 = mybir.dt.bfloat16
    Exp = mybir.ActivationFunctionType.Exp

    chunks = plan["chunks"]        # g -> [c...] ascending, 0 first
    qlo = plan["qlo"]              # (g,c) -> valid-q start col (0..511)
    win = plan["win"]              # (g,c) -> (mlo, mhi, tile_idx) or absent
    ntiles = plan["ntiles"]

    nc = bacc.Bacc("TRN2", target_bir_lowering=False, debug=False,
                   enable_asserts=False, num_devices=NCORES)

    # inputs are pre-tiled on the host so every DMA source is contiguous
    # per partition (fat descriptors: descriptor GENERATION on the Sync
    # engine, ~5ns each, is what gates the startup — not DMA bandwidth)
    xqT_d = nc.dram_tensor("xqT", [128, NG, 8, 512], bf16,
                           kind="ExternalInput").ap()
    xkvT_d = nc.dram_tensor("xkvT", [128, NG, 8, 512], bf16,
                            kind="ExternalInput").ap()
    wq_d = nc.dram_tensor("wq", [128, 8, 256], bf16, kind="ExternalInput").ap()
    wk_d = nc.dram_tensor("wk", [128, 8, 256], bf16, kind="ExternalInput").ap()
    wv_d = nc.dram_tensor("wv", [128, 8, 256], bf16, kind="ExternalInput").ap()
    wo_d = nc.dram_tensor("wo", [128, 2, 1024], bf16, kind="ExternalInput").ap()
    kprefT_d = nc.dram_tensor("kprefT", [128, 2, 128], bf16,
                              kind="ExternalInput").ap()
    vpref_d = nc.dram_tensor("vpref", [128, HPC, D], bf16, kind="ExternalInput").ap()
    if ntiles:
        maskblk_d = nc.dram_tensor("maskblk", [128, ntiles, 128], bf16,
                                   kind="ExternalInput").ap()
    outT_d = nc.dram_tensor("outT", [E, LQ], bf16, kind="ExternalOutput").ap()

    with tile.TileContext(nc) as tc, ExitStack() as stk:
        pers = stk.enter_context(tc.tile_pool(name="pers", bufs=1))

        def ptile(shape, name, dt=bf16):
            return pers.tile(shape, dt, tag=name, name=name)

        wq_sb = ptile([128, 8, 256], "wq_sb")
        wk_sb = ptile([128, 8, 256], "wk_sb")
        wv_sb = ptile([128, 8, 256], "wv_sb")
        wo_sb = ptile([128, 2, 1024], "wo_sb")
        # per-L-group tensors (512 wide); K prefix is its own [128,128] tile
        QTS = [[ptile([128, 512], f"QT{i}g{g}") for g in range(NG)] for i in range(2)]
        KTS = [[ptile([128, 512], f"KT{i}g{g}") for g in range(NG)] for i in range(2)]
        KPR = [ptile([128, 128], f"KP{i}") for i in range(2)]
        VTS = [ptile([128, HPC, 65], f"VT{c}") for c in range(NCH)]
        CTXT = [[ptile([128, 512], f"CTXT{i}g{g}") for g in range(NG)]
                for i in range(2)]
        cb0 = ptile([128, 1], "cb0", f32)
        ones_col = ptile([128, 64], "ones_col")
        mtall = ptile([128, max(ntiles, 1), 128], "mtall")

        def kslice(hc, c):
            if c == 0:
                return KPR[hc]
            g, off = (c - 1) // 4, 128 * ((c - 1) % 4)
            return KTS[hc][g][:, off:off + 128]

        xio = stk.enter_context(tc.tile_pool(name="xio", bufs=2))

        def proj_load(g, xq_t=None, xkv_t=None, halves=1):
            """x loads; `halves=2` splits per 4-ec half so the first
            projection matmuls gate on 0.5MB instead of 2MB (startup)."""
            if xq_t is None:
                xq_t = xio.tile([128, 8, 512], bf16, tag="xq", bufs=2, name="xq_t")
                xkv_t = xio.tile([128, 8, 512], bf16, tag="xkv", bufs=2,
                                 name="xkv_t")
            step = 8 // halves
            for t, t_d in ((xq_t, xqT_d), (xkv_t, xkvT_d)):
                for e0 in range(0, 8, step):
                    nc.sync.dma_start(out=t[:, e0:e0 + step, :],
                                      in_=t_d[:, g, e0:e0 + step, :])
            return xq_t, xkv_t

        # startup DMAs ordered by first consumption (q-t0 wq+xq, k-t0
        # wk+xkv, v xkv+wv); every source is per-partition contiguous so
        # descgen is ~128 descriptors per load
        xq0 = xio.tile([128, 8, 512], bf16, tag="xq", bufs=2, name="xq_t")
        xkv0 = xio.tile([128, 8, 512], bf16, tag="xkv", bufs=2, name="xkv_t")
        for e0 in (0, 4):
            nc.sync.dma_start(out=wq_sb[:, e0:e0 + 4, :], in_=wq_d[:, e0:e0 + 4, :])
            nc.sync.dma_start(out=xq0[:, e0:e0 + 4, :], in_=xqT_d[:, 0, e0:e0 + 4, :])
        nc.sync.dma_start(out=wk_sb, in_=wk_d)
        for e0 in (0, 4):
            nc.sync.dma_start(out=xkv0[:, e0:e0 + 4, :],
                              in_=xkvT_d[:, 0, e0:e0 + 4, :])
        nc.sync.dma_start(out=wv_sb, in_=wv_d)
        ld0 = (xq0, xkv0)

        nc.vector.memset(ones_col, 1.0)
        nc.vector.memset(cb0[0:64, :], 0.0)
        nc.vector.memset(cb0[64:128, :], NEG)
        for c in range(NCH):
            nc.vector.memset(VTS[c][:, :, 64:65], 1.0)
        for hc in range(2):
            nc.sync.dma_start(out=KPR[hc], in_=kprefT_d[:, hc, :])
        nc.sync.dma_start(out=VTS[0][:, :, 0:D], in_=vpref_d)
        if ntiles:
            nc.sync.dma_start(out=mtall, in_=maskblk_d)
        nc.sync.dma_start(out=wo_sb, in_=wo_d)

        attps = stk.enter_context(tc.tile_pool(name="att_ps", bufs=1, space="PSUM"))
        attsb = stk.enter_context(tc.tile_pool(name="att_sb", bufs=1))
        pjps = stk.enter_context(tc.tile_pool(name="pj_ps", bufs=1, space="PSUM"))

        # PE p-state warmup: the clock ramps only while the PE is busy, so
        # burn the DMA-wait window (engines boot ~7us, first real matmul's
        # data lands ~9us) on dummy matmuls; real work then starts warm.
        wps = pjps.tile([128, 512], f32, tag="pj", bufs=2, name="warm")
        for _ in range(72):
            nc.tensor.matmul(wps[0:64, 0:64], lhsT=ones_col[:, 0:64],
                             rhs=ones_col[:, 0:64], start=True, stop=True)

        def proj_units(g, loaded):
            """q/k/v projection for L-group g as a sequence of 4-matmul
            quanta (yield between quanta so attention can interleave)."""
            xq_t, xkv_t = loaded
            # each unit is atomic: its 8-matmul PSUM accumulation group must
            # close before anything else can allocate from the pj ring
            # q units first (both need only wq+xq, which arrive first at
            # startup); k units follow while wk/xkv are still streaming in
            for w_sb, x_t, dst in ((wq_sb, xq_t, QTS), (wk_sb, xkv_t, KTS)):
                for t in range(2):
                    ps = pjps.tile([128, 512], f32, tag="pj", bufs=2, name="ps_p")
                    for ec in range(8):
                        nc.tensor.matmul(
                            ps, lhsT=w_sb[:, ec, 128 * t:128 * t + 128],
                            rhs=x_t[:, ec, :], start=(ec == 0), stop=(ec == 7))
                    nc.vector.tensor_copy(out=dst[t][g], in_=ps)
                    yield
            for sub in range(4):
                ps = pjps.tile([128, 512], f32, tag="pj", bufs=2, name="ps_v")
                for ec in range(8):
                    nc.tensor.matmul(
                        ps[:, 0:256], lhsT=xkv_t[:, ec, 128 * sub:128 * sub + 128],
                        rhs=wv_sb[:, ec, :], start=(ec == 0), stop=(ec == 7))
                nc.vector.tensor_copy(
                    out=VTS[4 * g + sub + 1][:, :, 0:D],
                    in_=ps[:, 0:256].rearrange("p (h d) -> p h d", h=HPC))
                yield

        op_n = [0]

        def outproj_units(g):
            """out-projection for L-group g: 8 units of (2 matmuls + copy)."""
            gl = 512 * g
            for et in range(8):
                ops = pjps.tile([128, 512], f32, tag="pj", bufs=2, name="ops")
                for hc in range(2):
                    nc.tensor.matmul(
                        ops, lhsT=wo_sb[:, hc, 128 * et:128 * et + 128],
                        rhs=CTXT[hc][g], start=(hc == 0), stop=(hc == 1))
                ot = attsb.tile([128, 512], bf16, tag="ostage", bufs=6, name="ot")
                # PSUM->SBUF copy on DVE (ACT is saturated by the softmax
                # exps mid-run); the last group's units alternate onto ACT,
                # which is idle during the tail
                if g == NG - 1 and op_n[0] % 2 == 0:
                    nc.scalar.copy(ot, ops)
                else:
                    nc.vector.tensor_copy(out=ot, in_=ops)
                op_n[0] += 1
                nc.sync.dma_start(
                    out=outT_d[128 * et:128 * et + 128, gl:gl + 512], in_=ot)
                yield

        class FillerQ:
            """Two queues of generators (proj must flush at group
            boundaries, outproj carries over); step() advances one quantum,
            preferring proj."""
            def __init__(self):
                self.proj = []
                self.op = []

            @staticmethod
            def _step_one(q):
                while q:
                    try:
                        next(q[0])
                        return True
                    except StopIteration:
                        q.pop(0)
                return False

            def step(self, n=1):
                for _ in range(n):
                    if not self._step_one(self.proj):
                        self._step_one(self.op)

            def flush_proj(self):
                while self._step_one(self.proj):
                    pass

            def drain_all(self):
                self.flush_proj()
                while self._step_one(self.op):
                    pass

        fill = FillerQ()

        def denb_head(g, h, cx):
            """Per-head denominator finish (last group): partition-broadcast
            the raw denom row via a K=1 matmul, fast reciprocal (at
            partition offset 0 — the custom DVE op needs that), CTXT
            scaling on DVE."""
            hc, par = h // 2, h % 2
            # bf16 the raw denom row first: the K=1 broadcast matmul then
            # runs at 1 cycle/row instead of fp32's 4
            db = attsb.tile([1, 512], bf16, tag="db", bufs=2, name="db")
            nc.vector.tensor_copy(out=db, in_=cx[64:65, :])
            bc_ps = pjps.tile([128, 512], f32, tag="pj", bufs=2, name="bc_ps")
            nc.tensor.matmul(bc_ps[0:64, :], lhsT=ones_col[0:1, :],
                             rhs=db, start=True, stop=True)
            rcs = attsb.tile([64, 512], f32, tag="rcs", bufs=2, name="rcs")
            nc.vector.reciprocal_approx_fast(out=rcs, in_=bc_ps[0:64, :])
            if par == 0:
                nc.vector.tensor_mul(CTXT[hc][g][0:64, :], cx[0:64, :], rcs)
            else:
                st = attsb.tile([64, 512], bf16, tag="stage", bufs=2, name="st")
                nc.vector.tensor_mul(st, cx[0:64, :], rcs)
                nc.sync.dma_start(out=CTXT[hc][g][64:128, :], in_=st)

        def emit_scores(g, h, batch):
            """Score matmuls + exp for one chunk batch; returns the pr tile."""
            hc, prow = h // 2, 64 * (h % 2)
            sc = attps.tile([128, 1024], f32, tag="sc", bufs=2, name=f"sc{h}")
            for j, c in enumerate(batch):
                q0 = qlo[(g, c)]
                # second chunk of a pair starts at its 512 boundary (q0=0)
                # even when causally trimmable: the few extra columns are
                # harmless garbage scores and keep the whole exp range of
                # the single per-batch activation initialized
                m0 = q0 if j == 0 else 0
                nc.tensor.matmul(
                    sc[:, 512 * j + m0:512 * j + 512],
                    lhsT=kslice(hc, c)[prow:prow + 64, :],
                    rhs=QTS[hc][g][prow:prow + 64, m0:512],
                    start=True, stop=True)
            pr = attsb.tile([128, 1024], bf16, tag="pr", bufs=4, name=f"pr{h}")
            e0 = qlo[(g, batch[0])]
            we = 512 * len(batch)
            if batch[0] == 0:
                nc.scalar.activation(pr[:, e0:we], sc[:, e0:we], Exp,
                                     bias=cb0[:, 0:1])
            else:
                nc.scalar.activation(pr[:, e0:we], sc[:, e0:we], Exp)
            return pr

        def attn_group(g, head_hook=None, pending_cb=None, tail_cb=None):
            """Score/softmax/ctx for group g, software-pipelined one batch
            ahead: batch b+1's score matmuls sit between exp(b) and ctx(b)
            in the PE stream, so the in-order PE queue always has real work
            covering the softmax (ACT) latency.  The denominator finish is
            entirely off-PE (GpSimd broadcast + DVE reciprocal/scale), so
            group boundaries are seamless for the PE."""
            cs = chunks[g]
            batches = [[cs[0]]] + [cs[1 + i:3 + i] for i in range(0, len(cs) - 1, 2)]
            nbat = len(batches)
            last = (g == NG - 1)
            ctxs = {}
            if not last:
                denoms4 = attsb.tile([97, 512], f32, tag="den4", bufs=2,
                                     name="denoms4")
                nc.vector.memset(denoms4, 1.0)
            # last group runs par==1 heads (whose CTXT write needs an extra
            # SBUF->SBUF DMA hop) first, so the tail head finishes directly
            pend_denb = None
            for hi, h in enumerate([1, 3, 2, 0] if last else range(HPC)):
                if head_hook is not None:
                    head_hook()
                ctx_ps = attps.tile([65, 512], f32, tag="ctx", bufs=2,
                                    name=f"ctx{h}")
                prs = {0: emit_scores(g, h, batches[0])}
                if pend_denb is not None:
                    # previous head's denb, emitted one head late so its
                    # broadcast matmul never stalls the PE queue head
                    pend_denb()
                    pend_denb = None
                for bi, batch in enumerate(batches):
                    if bi + 1 < nbat:
                        prs[bi + 1] = emit_scores(g, h, batches[bi + 1])
                    fill.step(1)
                    if pending_cb and not (hi == 0 and bi == 0):
                        # one previous-group finish part per batch: the PE
                        # reaches each broadcast a full batch after its DVE
                        # producer was emitted, so it never stalls
                        pending_cb.pop(0)()
                    pr = prs.pop(bi)
                    for j, c in enumerate(batch):
                        for mlo, mhi, ti in win.get((g, c), ()):
                            nc.vector.tensor_mul(
                                pr[:, 512 * j + mlo:512 * j + mhi],
                                pr[:, 512 * j + mlo:512 * j + mhi],
                                mtall[:, ti, 0:mhi - mlo])
                    for j, c in enumerate(batch):
                        q0 = qlo[(g, c)]
                        nc.tensor.matmul(
                            ctx_ps[:, q0:512],
                            lhsT=VTS[c][:, h, :],
                            rhs=pr[:, 512 * j + q0:512 * j + 512],
                            start=(bi == 0 and j == 0),
                            stop=(bi == nbat - 1 and j == len(batch) - 1))
                    fill.step(1)
                # copy ctx+denom to SBUF to release the PSUM bank
                ctxs[h] = attsb.tile([65, 512], f32, tag="ctxs", bufs=8,
                                     name=f"ctxs{h}")
                nc.vector.tensor_copy(out=ctxs[h], in_=ctx_ps)
                if last:
                    # last group: per-head denominator finish (deferred one
                    # head; the tail head emits immediately, preceded by the
                    # reserved outproj units so the PE chews them while the
                    # denb's DVE chain runs instead of idling at the bc mm)
                    if h == 0:
                        if tail_cb is not None:
                            tail_cb()
                        denb_head(g, h, ctxs[h])
                    else:
                        pend_denb = (lambda hh=h: denb_head(g, hh, ctxs[hh]))
                else:
                    # gather the denom row into denoms4 at partition 32h
                    # for the batched per-group reciprocal
                    nc.sync.dma_start(out=denoms4[32 * h:32 * h + 1, :],
                                      in_=ctxs[h][64:65, :])
            while pending_cb:
                pending_cb.pop(0)()
            if last:
                return None

            # batched denominator finish, returned as SPREAD parts: one
            # reciprocal over the four gathered denom rows, then per head a
            # K=1 broadcast matmul + DVE scale.  The caller feeds one part
            # per batch of the next group so the in-order PE queue reaches
            # each broadcast well after its DVE producer.
            state = {}

            def recip_part():
                rc4 = attsb.tile([97, 512], f32, tag="rc4", bufs=2, name="rc4")
                nc.vector.reciprocal_approx_fast(out=rc4, in_=denoms4)
                rc4b = attsb.tile([97, 512], bf16, tag="rc4b", bufs=2,
                                  name="rc4b")
                nc.vector.tensor_copy(out=rc4b, in_=rc4)
                state["rc4b"] = rc4b

            def bc_part(h):
                hc, par = h // 2, h % 2
                rc4b = state["rc4b"]
                bc_ps = pjps.tile([128, 512], f32, tag="pj", bufs=2,
                                  name="bc_ps")
                nc.tensor.matmul(bc_ps[0:64, :],
                                 lhsT=ones_col[32 * h:32 * h + 1, :],
                                 rhs=rc4b[32 * h:32 * h + 1, :],
                                 start=True, stop=True,
                                 tile_position=(32 * h, 0))
                if par == 0:
                    nc.vector.tensor_mul(CTXT[hc][g][0:64, :],
                                         ctxs[h][0:64, :], bc_ps[0:64, :])
                else:
                    st = attsb.tile([64, 512], bf16, tag="stage", bufs=2,
                                    name="st")
                    nc.vector.tensor_mul(st, ctxs[h][0:64, :],
                                         bc_ps[0:64, :])
                    nc.sync.dma_start(out=CTXT[hc][g][64:128, :], in_=st)
            return [recip_part] + [lambda h=h: bc_part(h) for h in range(HPC)]

        # schedule: proj(0) upfront; during attn(g) the filler queue holds
        # proj(g+1) plus outproj(g-1) (whose CTXT finished, off-PE, at the
        # end of attn(g-1)); proj leftovers flush at the group boundary
        # (they gate attn(g+1)); op leftovers carry over and also cover the
        # final denb chain before the outproj(3) tail.
        def take(gen, n):
            def taken():
                for _ in range(n):
                    try:
                        next(gen)
                    except StopIteration:
                        return
                    yield
            return taken()

        for _ in proj_units(0, ld0):
            pass
        pending = []
        for g in range(NG):
            if g + 1 < NG:
                ld = proj_load(g + 1)
                fill.proj.append(proj_units(g + 1, ld))
            if g == NG - 1:
                # ration outproj(2): 2 units at the 2nd/3rd head starts
                # (after the finish(2) parts drained), 4 held back so the
                # PE still has work queued behind the final ctx matmuls
                # while the tail denb chain runs
                op2 = outproj_units(g - 1)
                feed = iter([0, 2, 2, 0])

                def hook():
                    n = next(feed)
                    if n:
                        fill.op.append(take(op2, n))

                def tail_cb():
                    for _ in op2:
                        pass
                attn_group(g, head_hook=hook, pending_cb=pending,
                           tail_cb=tail_cb)
            else:
                parts = attn_group(g, pending_cb=pending)
                fill.flush_proj()
                pending = parts
                if g + 1 < NG - 1:
                    # after the last finish part: outproj(g) becomes
                    # available (emission order guarantees op reads CTXT
                    # after the finish writes it)
                    pending = parts + [
                        lambda g=g: fill.op.append(outproj_units(g))]
        fill.drain_all()
        for _ in outproj_units(NG - 1):
            pass

    nc.compile()
    return nc


def _make_plan(mask):
    """Block plan from the actual mask (union over batches -> one SPMD plan).

    For each (q-group g, kv-chunk c) computes:
      - inclusion (any valid element),
      - qlo: first q column (within the group's 512) with any valid kv,
      - the mixed window [mlo, mhi) of q columns that need an elementwise
        mask multiply, with deduped [128, mhi-mlo<=128...] tiles.
    """
    m = np.asarray(mask[:, 0])                       # [B, LQ, LKV] bool
    chunks, qlo, wins = [], {}, {}
    uniq, order = {}, []                             # content-hash -> idx
    for g in range(NG):
        cl = [0]
        qlo[(g, 0)] = 0
        for c in range(1, NCH):
            blk = m[:, 512 * g:512 * g + 512, 128 * (c - 1):128 * c]  # [B,512,128]
            anyk = blk.any(axis=2)                   # [B, 512]
            if not anyk.any():
                continue
            cl.append(c)
            valid_cols = anyk.any(axis=0)            # union over batches
            q0 = int(np.argmax(valid_cols))
            qlo[(g, c)] = q0
            allk = blk.all(axis=2).all(axis=0)       # [512] all-valid cols
            mixed = valid_cols & ~allk
            if mixed.any():
                lo = int(np.argmax(mixed))
                hi = 512 - int(np.argmax(mixed[::-1]))
                wl = []
                # split into <=128-wide windows (mask tiles are [128,128])
                for mlo in range(lo, hi, 128):
                    mhi = min(mlo + 128, hi)
                    # per-batch tile content; dedup on the union key so all
                    # cores run the same program with per-core data
                    key = (mhi - mlo, blk[:, mlo:mhi, :].tobytes())
                    if key not in uniq:
                        uniq[key] = len(order)
                        order.append((g, c, mlo, mhi))
                    wl.append((mlo, mhi, uniq[key]))
                wins[(g, c)] = wl
        chunks.append(cl)
    return {"chunks": chunks, "qlo": qlo, "win": wins, "ntiles": len(order),
            "order": order}


def _prep_core_inputs(inputs, plan):
    """Per-core input dicts (8 cores: batch-major, then head-group)."""
    inputs_q = np.asarray(inputs["inputs_q"], dtype=np.float32)
    inputs_kv = np.asarray(inputs["inputs_kv"], dtype=np.float32)
    key_prefix = np.asarray(inputs["key_prefix"], dtype=np.float32)
    value_prefix = np.asarray(inputs["value_prefix"], dtype=np.float32)
    mask = np.asarray(inputs["mask"])
    Wq = np.asarray(inputs["Wq"], dtype=np.float32)
    Wk = np.asarray(inputs["Wk"], dtype=np.float32)
    Wv = np.asarray(inputs["Wv"], dtype=np.float32)
    Wo = np.asarray(inputs["Wo"], dtype=np.float32)

    def tile_x(x):
        # [L, E] -> [E, L] -> [128p, NG, 8ec, 512]: per-partition contiguous
        xT = x.T.reshape(8, 128, NG, 512).transpose(1, 2, 0, 3)
        return np.ascontiguousarray(xT.astype(BF16))

    def tile_w(w):
        # [E, 256] -> [128p, 8ec, 256]
        return np.ascontiguousarray(
            w.reshape(8, 128, HPC * D).transpose(1, 0, 2).astype(BF16))

    xT = [tile_x(inputs_q[b]) for b in range(B)]
    xkT = [tile_x(inputs_kv[b]) for b in range(B)]

    maskblks = []
    ntiles = plan["ntiles"]
    for b in range(B):
        mb = np.zeros((max(ntiles, 1), 128, 128), np.float32)
        for i, (g, c, mlo, mhi) in enumerate(plan["order"]):
            mb[i, :, 0:mhi - mlo] = mask[
                b, 0, 512 * g + mlo:512 * g + mhi,
                128 * (c - 1):128 * c].T.astype(np.float32)
        # -> [128p, ntiles, 128]: per-partition contiguous
        maskblks.append(np.ascontiguousarray(
            mb.transpose(1, 0, 2).astype(BF16)))

    in_maps = []
    for core in range(NCORES):
        b, hg = core // HGROUPS, core % HGROUPS
        hs = slice(HPC * hg, HPC * (hg + 1))
        kpT = key_prefix[b, :, hs, :]                 # [P, 4, D]
        kpT = kpT.transpose(1, 2, 0).reshape(2, 128, P)  # [hc, (2 heads x D), P]
        kpT = np.concatenate(
            [kpT, np.zeros((2, 128, 128 - P), np.float32)], axis=2)
        im = {
            "xqT": xT[b],
            "xkvT": xkT[b],
            "wq": tile_w((Wq[:, hs, :] / np.sqrt(D)).reshape(E, HPC * D)),
            "wk": tile_w(Wk[:, hs, :].reshape(E, HPC * D)),
            "wv": tile_w(Wv[:, hs, :].reshape(E, HPC * D)),
            "wo": np.ascontiguousarray(
                Wo[hs].reshape(2, 128, E).transpose(1, 0, 2).astype(BF16)),
            "kprefT": np.ascontiguousarray(
                kpT.transpose(1, 0, 2).astype(BF16)),
            "vpref": np.ascontiguousarray(np.concatenate(
                [value_prefix[b, :, hs, :],
                 np.zeros((128 - P, HPC, D), np.float32)], axis=0).astype(BF16)),
        }
        if ntiles:
            im["maskblk"] = maskblks[b]
        in_maps.append(im)
    return in_maps


def kernel(**inputs) -> np.ndarray:
    from concourse import bass_utils

    plan = _make_plan(inputs["mask"])
    key = (tuple(tuple(c) for c in plan["chunks"]),
           tuple(sorted(plan["qlo"].items())),
           tuple(sorted((k, tuple(v)) for k, v in plan["win"].items())),
           plan["ntiles"])
    if key not in _CACHE:
        _CACHE[key] = _build_module(plan)
    nc = _CACHE[key]

    in_maps = _prep_core_inputs(inputs, plan)
    res = bass_utils.run_bass_kernel_spmd(nc, in_maps, core_ids=list(range(NCORES)))

    out = np.zeros((B, LQ, E), np.float32)
    for core in range(NCORES):
        b = core // HGROUPS
        out[b] += res.results[core]["outT"].T.astype(np.float32)
    return out

